# revision 26
# baseline (speedup 1.0000x reference)
"""Trainium2 Bass kernel for nn_MetaLEAPPredictor (GNN edge scoring).

reference:
    w0   = sf @ psi_w.T + psi_b                      # [E, 2C]
    coef = w0 + delta_w[li] + u[li]
    s    = sum(x[row] * coef[:, :C], -1) + sum(x[col] * coef[:, C:], -1)
    y    = gamma_h[li][None, :] * leaky_relu(s, 0.01)[:, None]

Algebraic restructure: with b0 = psi_b + delta_w[li] + u[li],
    s_e = <[sf_e, 1], T[row_e, 0:5]> + <[sf_e, 1], T[col_e, 8:13]>
where T = x @ W16, W16[c, 0:4] = psi_w[c, :4], W16[c, 4] = b0[c],
W16[c, 8:12] = psi_w[64+c, :], W16[c, 12] = b0[64+c]   (c in [0, 64)).

Device plan (8 cores, edges sharded; per core):
  Phase A (full node set, t-major table): cast x to bf16 (SWDGE cast DMA),
  DMA-transpose packed node pairs, PE matmul vs W16 (bf16) into 16-col
  PSUM, Act copies PSUM->SBUF, sync writes table rows (t*128+perm(p))
  cols 0:16 of an [NP, 64] f32 table (256B row stride for the gather).
  Table completion tracked per 32768-row chunk (s_chunk sems).
  Phase B: edges bucketed by (row chunk, col chunk) and sorted by row
  index; per 1024-edge tile, two InstDMAGatherAnt fetches (row/col) of
  64B table rows; DVE does mul/reduce/leaky/broadcast; GPSIMD stores y.
  Gathers are gated per-bucket on the chunks they read, overlapping
  phase A and phase B.
Synchronization uses per-buffer-slot semaphores with exact counts: a
wait threshold always equals the total of every increment ever issued
on that semaphore at that point, so out-of-order sub-increment delivery
from the 16 SDMA engines cannot release a wait early.
"""
import sys
if '/opt/trn_rl_repo' not in sys.path:
    sys.path.insert(0, '/opt/trn_rl_repo')

import numpy as np
import ml_dtypes

import concourse.bacc as bacc
import concourse.bass as bass
import concourse.mybir as mybir
from concourse import ap_utils
from concourse.bass import exact_div, round_up_to_multiple
from concourse.library_config import mlp
from concourse.bass_utils import run_bass_kernel_spmd

N = 100000
C = 64
E = 1600000
H = 8
NEG = 0.01
NCORES = 8
NP = 100096            # N padded to 128*782
TILES = NP // 128      # 782
CHUNK = 32768          # gather chunk (int16 range)
NCHUNKS = 4            # ceil(100096 / 32768)
GT = 2048              # edges per gather instruction
GSMAX = 8              # gather tiles per group
SL = GT // 128
IW = GT // 16
PT = 8                 # phase-A tiles per group
NGA = (TILES + PT - 1) // PT          # 98
NCAST = 8
LSLOT = 6              # load buffer slots
CSLOT = 4              # gather/compute buffer slots
# chunk k complete after phase-A groups g < CHG[k]
CHG = [32, 64, 96, NGA]
CHUNK_W = [32, 32, 32, NGA - 96]      # table writes per chunk per rep


def _dma_gather_raw(gp, out_ap, in_ap, idxs_ap, num_idxs, elem_size, elem_step,
                    queue_num=0):
    """bass.dma_gather minus the (transpose-only) elem%256 assert."""
    assert idxs_ap.dtype == mybir.dt.int16
    assert in_ap.dtype == out_ap.dtype
    assert in_ap.space == bass.MemorySpace.DRAM
    assert ap_utils.ap_is_contiguous(out_ap.ap[1:])
    assert ap_utils.ap_is_contiguous(idxs_ap.ap[1:])
    assert in_ap.ap[-1][1] == out_ap.ap[-1][1] == elem_size
    assert out_ap.ap[0][1] * out_ap.ap[1][1] == round_up_to_multiple(num_idxs, 128)
    assert in_ap.ap[0][0] == elem_step
    stride_bytes_256 = exact_div(elem_step * mybir.dt.size(in_ap.dtype), 256)
    _in_ap = gp.lower_ap_dma(in_ap, for_custom_bir_dma=True)
    return gp.add_instruction(
        mybir.InstDMAGatherAnt(
            name=gp.bass.get_next_instruction_name(),
            ins=[*_in_ap, gp.lower_ap(idxs_ap),
                 gp.lower_val_access(gp.to_reg(num_idxs))],
            outs=[gp.lower_ap(out_ap)],
            transpose=False, num_idxs=num_idxs, elem_size=elem_size,
            stride_bytes_256=stride_bytes_256, gen_mode=0, single_packet=False,
            queue_num=queue_num, sbuf_tokens_per_rank=0, sbuf_free_dim_per_rank=0,
            sbuf_free_dim_pad_per_rank=0, sbuf_byte_offset=0,
        ))


def build_program(groups, nrep=1, a_once=False):
    """groups: (bucket, gs, nis); nis = per-tile num_idxs (mult of 128)."""
    NG = len(groups)

    # ---- planning: exact semaphore totals -------------------------------
    ld_after, gab_after, st_after = [], [], []
    ld_c = [0] * LSLOT
    gab_c = [[0] * 4 for _ in range(CSLOT)]
    st_c = [0] * CSLOT
    for rep in range(nrep):
        for G in range(NG):
            gidx = rep * NG + G
            gs = groups[G][1]
            ld_c[gidx % LSLOT] += 48
            ld_after.append(ld_c[gidx % LSLOT])
            for t in range(gs):
                gab_c[gidx % CSLOT][(2 * t) % 4] += 16
                gab_c[gidx % CSLOT][(2 * t + 1) % 4] += 16
            gab_after.append(tuple(gab_c[gidx % CSLOT]))
            st_c[gidx % CSLOT] += 16
            st_after.append(st_c[gidx % CSLOT])
    na_rep = 1 if a_once else nrep
    tr_after = []
    tr_c = [0] * 3
    for gg in range(na_rep * NGA):
        tr_c[gg % 3] += 16
        tr_after.append(tr_c[gg % 3])

    nc = bacc.Bacc("TRN2", target_bir_lowering=False, debug=False,
                   num_devices=NCORES, num_swdge_queues=4,
                   detect_race_conditions=False,
                   dynamic_dma_scratch_size=65536)

    w16 = nc.dram_tensor("w16", [128, 16], mybir.dt.bfloat16,
                         kind="ExternalInput")
    gamma = nc.dram_tensor("gamma", [128, H], mybir.dt.float32,
                           kind="ExternalInput")
    idxr = nc.dram_tensor("idxr", [NG, 128, GSMAX * IW], mybir.dt.int16,
                          kind="ExternalInput")
    idxc = nc.dram_tensor("idxc", [NG, 128, GSMAX * IW], mybir.dt.int16,
                          kind="ExternalInput")
    sfd = nc.dram_tensor("sfd", [NG, 128, GSMAX * SL * 4], mybir.dt.float32,
                         kind="ExternalInput")
    ydev = nc.dram_tensor("ydev", [NG, 128, GSMAX * SL * H], mybir.dt.float32,
                          kind="ExternalOutput")
    xbf = nc.dram_tensor("xbf", [NP * C], mybir.dt.bfloat16,
                         kind="ExternalInput")
    table = nc.dram_tensor("table", [NP, 64], mybir.dt.float32)

    xbf_pack = xbf[:].rearrange("(r c) -> r c", c=2 * C)
    # table row = t*128 + p; write element (p, j, c) -> ((t0+j)*128+p)*64 + c
    table_v = table[:].rearrange("(t p) c -> p t c", p=128)

    rows_per = (N + NCAST - 1) // NCAST
    PADE = (NP - N) * C // 128

    import contextlib
    with contextlib.ExitStack() as ctx:
        e = ctx.enter_context
        wt = e(nc.sbuf_tensor("wt", [128, 16], mybir.dt.bfloat16))
        gm = e(nc.sbuf_tensor("gm", [128, H], mybir.dt.float32))
        xts = [e(nc.sbuf_tensor(f"xt{i}", [128, PT * 64], mybir.dt.bfloat16))
               for i in range(3)]
        stg = [e(nc.sbuf_tensor(f"stg{i}", [128, PT * 16], mybir.dt.float32))
               for i in range(32)]
        psb = [e(nc.psum_tensor(f"ps{i}", [128, PT * 16], mybir.dt.float32))
               for i in range(4)]
        irs = [e(nc.sbuf_tensor(f"ir{i}", [128, GSMAX * IW], mybir.dt.int16))
               for i in range(LSLOT)]
        ics = [e(nc.sbuf_tensor(f"ic{i}", [128, GSMAX * IW], mybir.dt.int16))
               for i in range(LSLOT)]
        sfs = [e(nc.sbuf_tensor(f"sf{i}", [128, GSMAX * SL * 4],
                                mybir.dt.float32)) for i in range(LSLOT)]
        grs = [e(nc.sbuf_tensor(f"gr{i}", [128, GSMAX, SL, 16],
                                mybir.dt.float32)) for i in range(CSLOT)]
        gcs = [e(nc.sbuf_tensor(f"gc{i}", [128, GSMAX, SL, 16],
                                mybir.dt.float32)) for i in range(CSLOT)]
        yts = [e(nc.sbuf_tensor(f"yt{i}", [128, GSMAX * SL, H],
                                mybir.dt.float32)) for i in range(CSLOT)]
        scs = [e(nc.sbuf_tensor(f"sc{i}", [128, GSMAX * SL, 6],
                                mybir.dt.float32)) for i in range(CSLOT)]
        s_ms = e(nc.semaphore("s_ms"))
        s_tr = [e(nc.semaphore(f"s_tr{i}")) for i in range(3)]
        s_pe = e(nc.semaphore("s_pe"))
        s_cp = e(nc.semaphore("s_cp"))
        s_chunk = [e(nc.semaphore(f"s_chunk{i}")) for i in range(NCHUNKS)]
        s_ld = [e(nc.semaphore(f"s_ld{i}")) for i in range(LSLOT)]
        s_gab = [[e(nc.semaphore(f"s_gab{i}_{q}")) for q in range(4)]
                 for i in range(CSLOT)]
        s_cmp = e(nc.semaphore("s_cmp"))
        s_v = e(nc.semaphore("s_v"))
        s_gz = e(nc.semaphore("s_gz"))
        s_st = [e(nc.semaphore(f"s_st{i}")) for i in range(CSLOT)]
        block = e(nc.Block())

        @block.sync
        def _(sy):
            sy.dma_start(wt[:], w16[:]).then_inc(s_ms, 16)
            sy.dma_start(gm[:], gamma[:]).then_inc(s_ms, 16)
            for rep in range(na_rep):
                for g in range(NGA):
                    gg = rep * NGA + g
                    t0 = g * PT
                    nt = min(PT, TILES - t0)
                    prows = nt * 64
                    if gg >= 3:
                        sy.wait_ge(s_pe, gg - 2)
                    sy.dma_start_transpose(
                        xts[gg % 3][:, :prows],
                        xbf_pack[t0 * 64: t0 * 64 + prows, :]
                    ).then_inc(s_tr[gg % 3], 16)
                    if g >= 1:
                        gp_ = g - 1
                        sy.wait_ge(s_cp, rep * NGA + gp_ + 1)
                        tp0 = gp_ * PT
                        ntp = min(PT, TILES - tp0)
                        sy.dma_start(
                            table_v[:, tp0:tp0 + ntp, 0:16],
                            stg[gp_ % 32][:, :ntp * 16].rearrange(
                                "p (t c) -> p t c", c=16)
                        ).then_inc(s_chunk[gp_ // 32], 16)
                g = NGA - 1
                sy.wait_ge(s_cp, rep * NGA + g + 1)
                tp0 = g * PT
                ntp = min(PT, TILES - tp0)
                sy.dma_start(
                    table_v[:, tp0:tp0 + ntp, 0:16],
                    stg[g % 32][:, :ntp * 16].rearrange("p (t c) -> p t c", c=16)
                ).then_inc(s_chunk[g // 32], 16)

        @block.gpsimd
        def _(gp):
            gp.load_library(mlp)
            gp.wait_ge(s_gz, 1)
            chunk_seen = -1
            for rep in range(nrep):
                if rep < na_rep:
                    chunk_seen = -1
                for G in range(NG):
                    gidx = rep * NG + G
                    b, gs = groups[G][0], groups[G][1]
                    rc, cc = divmod(b, NCHUNKS)
                    need = max(rc, cc)
                    while chunk_seen < need:
                        chunk_seen += 1
                        gp.wait_ge(s_chunk[chunk_seen],
                                   CHUNK_W[chunk_seen] * 16 *
                                   (min(rep, na_rep - 1) + 1))
                    if gidx >= 3:
                        gp.wait_ge(s_cmp, gidx - 2)
                    gp.wait_ge(s_ld[gidx % LSLOT], ld_after[gidx])
                    rlo, clo = rc * CHUNK, cc * CHUNK
                    src_r = table[rlo:min(NP, rlo + CHUNK), 0:16]
                    src_c = table[clo:min(NP, clo + CHUNK), 0:16]
                    sl3 = gidx % CSLOT
                    nis = groups[G][2]
                    for t in range(gs):
                        ni = nis[t]
                        _dma_gather_raw(
                            gp, grs[sl3][:, t, 0:ni // 128], src_r,
                            irs[gidx % LSLOT][:, t * IW:t * IW + ni // 16],
                            ni, 16, 64,
                            queue_num=(2 * t) % 4
                            ).then_inc(s_gab[sl3][(2 * t) % 4], 16)
                        _dma_gather_raw(
                            gp, gcs[sl3][:, t, 0:ni // 128], src_c,
                            ics[gidx % LSLOT][:, t * IW:t * IW + ni // 16],
                            ni, 16, 64,
                            queue_num=(2 * t + 1) % 4
                            ).then_inc(s_gab[sl3][(2 * t + 1) % 4], 16)

        @block.scalar
        def _(ac):
            def store(rep, G):
                gidx = rep * NG + G
                gs = groups[G][1]
                nv = gs * SL
                ac.wait_ge(s_cmp, gidx + 1)
                ac.dma_start(
                    ydev[G, :, :nv * H],
                    yts[gidx % CSLOT][:, :nv].rearrange("p s h -> p (s h)"),
                ).then_inc(s_st[gidx % CSLOT], 16)

            def loads(rep, G):
                gidx = rep * NG + G
                gs = groups[G][1]
                sl = gidx % LSLOT
                ac.dma_start(irs[sl][:, :gs * IW],
                             idxr[G, :, :gs * IW]).then_inc(s_ld[sl], 16)
                ac.dma_start(ics[sl][:, :gs * IW],
                             idxc[G, :, :gs * IW]).then_inc(s_ld[sl], 16)
                ac.dma_start(sfs[sl][:, :gs * SL * 4],
                             sfd[G, :, :gs * SL * 4]).then_inc(s_ld[sl], 16)

            for rep in range(nrep):
                for G in range(min(LSLOT, NG)):
                    loads(rep, G)
                for g in range(NGA if rep < na_rep else 0):
                    gg = rep * NGA + g
                    nt = min(PT, TILES - g * PT)
                    ac.wait_ge(s_pe, gg + 1)
                    if g >= 32:
                        k = g // 32 - 1
                        ac.wait_ge(s_chunk[k], CHUNK_W[k] * 16 * (rep + 1))
                    elif rep > 0:
                        k = 3 if g < NGA - 96 else 2
                        ac.wait_ge(s_chunk[k], CHUNK_W[k] * 16 * rep)
                    ac.copy(stg[g % 32][:, :nt * 16],
                            psb[gg % 4][:, :nt * 16]).then_inc(s_cp, 1)
                for G in range(NG):
                    store(rep, G)
                    if G + LSLOT < NG:
                        loads(rep, G + LSLOT)

        @block.tensor
        def _(te):
            te.wait_ge(s_ms, 32)
            for rep in range(na_rep):
                for g in range(NGA):
                    gg = rep * NGA + g
                    t0 = g * PT
                    nt = min(PT, TILES - t0)
                    te.wait_ge(s_tr[gg % 3], tr_after[gg])
                    if gg >= 4:
                        te.wait_ge(s_cp, gg - 3)
                    ps = psb[gg % 4]
                    xt = xts[gg % 3]
                    last = None
                    for j in range(nt):
                        o = j * 64
                        te.matmul(ps[0:64, j * 16:(j + 1) * 16],
                                  xt[0:C, o:o + 64],
                                  wt[0:C, :], start=True, stop=True)
                        last = te.matmul(ps[64:128, j * 16:(j + 1) * 16],
                                         xt[C:2 * C, o:o + 64],
                                         wt[C:2 * C, :], start=True, stop=True)
                    last.then_inc(s_pe, 1)

        @block.vector
        def _(ve):
            bufs = grs + gcs
            for buf in bufs[:-1]:
                ve.memset(buf[:], 0.0)
            ve.memset(bufs[-1][:], 0.0).then_inc(s_gz, 1)
            ve.wait_ge(s_ms, 32)
            vctr = [0]

            def chain(inst):
                vctr[0] += 1
                inst.then_inc(s_v, 1)
                ve.wait_ge(s_v, vctr[0])
            for rep in range(nrep):
                for G in range(NG):
                    gidx = rep * NG + G
                    b, gs = groups[G][0], groups[G][1]
                    nv = gs * SL
                    sl3 = gidx % CSLOT
                    sll = gidx % LSLOT
                    for q in range(4):
                        if gab_after[gidx][q]:
                            ve.wait_ge(s_gab[sl3][q], gab_after[gidx][q])
                    ve.wait_ge(s_ld[sll], ld_after[gidx])
                    if gidx >= CSLOT:
                        ve.wait_ge(s_st[sl3], st_after[gidx - CSLOT])
                    sf4 = sfs[sll][:, :nv * 4].rearrange("p (s k) -> p s k", k=4)
                    grv = grs[sl3][:, :gs].rearrange("p t s e -> p (t s) e")
                    gcv = gcs[sl3][:, :gs].rearrange("p t s e -> p (t s) e")
                    yt = yts[sl3]
                    sc = scs[sl3]
                    pr = sc[:, :nv, 0:4]
                    s0 = sc[:, :nv, 4:5].squeeze(2)
                    s1 = sc[:, :nv, 5:6].squeeze(2)
                    chain(ve.tensor_tensor(out=pr, in0=sf4, in1=grv[:, :, 0:4],
                                           op=mybir.AluOpType.mult))
                    chain(ve.tensor_reduce(out=s0, in_=pr,
                                           axis=mybir.AxisListType.X,
                                           op=mybir.AluOpType.add))
                    chain(ve.tensor_tensor(out=pr, in0=sf4, in1=gcv[:, :, 8:12],
                                           op=mybir.AluOpType.mult))
                    chain(ve.tensor_reduce(out=s1, in_=pr,
                                           axis=mybir.AxisListType.X,
                                           op=mybir.AluOpType.add))
                    chain(ve.tensor_tensor(out=s0, in0=s0, in1=s1,
                                           op=mybir.AluOpType.add))
                    chain(ve.tensor_tensor(out=s0, in0=s0,
                                           in1=grv[:, :, 4:5].squeeze(2),
                                           op=mybir.AluOpType.add))
                    chain(ve.tensor_tensor(out=s0, in0=s0,
                                           in1=gcv[:, :, 12:13].squeeze(2),
                                           op=mybir.AluOpType.add))
                    chain(ve.scalar_tensor_tensor(out=s0, in0=s0, scalar=NEG,
                                                  in1=s0,
                                                  op0=mybir.AluOpType.mult,
                                                  op1=mybir.AluOpType.max))
                    ve.tensor_tensor(
                        out=yt[:, :nv],
                        in0=s0.unsqueeze(2).broadcast_to([128, nv, H]),
                        in1=gm[:].unsqueeze(1).broadcast_to([128, nv, H]),
                        op=mybir.AluOpType.mult).then_inc(s_cmp, 1)

    nc.compile()
    return nc


def _trow(n):
    """t-major table row: row = (n//128)*128 + perm(n%128)."""
    m = n % 128
    return (n // 128) * 128 + (m % 2) * 64 + m // 2


def _bucket_order():
    order = sorted(range(NCHUNKS * NCHUNKS),
                   key=lambda b: (max(b // NCHUNKS, b % NCHUNKS),
                                  b // NCHUNKS, b % NCHUNKS))
    return order


SORT_MODE = "row"


def prep_inputs(x, edge_index, structural_features, layer_idx,
                psi_w, psi_b, delta_w, u, gamma_h):
    psi_w = np.asarray(psi_w, np.float32)
    psi_b = np.asarray(psi_b, np.float32)
    delta_w = np.asarray(delta_w, np.float32)
    u = np.asarray(u, np.float32)
    gamma_h = np.asarray(gamma_h, np.float32)
    li = int(layer_idx)
    b0 = (psi_b + delta_w[li] + u[li]).astype(np.float32)       # [2C]
    w16 = np.zeros((128, 16), dtype=np.float32)
    w16[0:C, 0:4] = psi_w[:C]
    w16[0:C, 4] = b0[:C]
    w16[0:C, 8:12] = psi_w[C:]
    w16[0:C, 12] = b0[C:]
    w16[C:128] = w16[0:C]
    w16 = w16.astype(ml_dtypes.bfloat16)
    gamma = np.tile(np.asarray(gamma_h[li], np.float32)[None, :], (128, 1))

    xbf_host = np.zeros(NP * C, dtype=ml_dtypes.bfloat16)
    xbf_host[:N * C] = np.asarray(x, np.float32).astype(
        ml_dtypes.bfloat16).ravel()

    row = np.asarray(edge_index[0], np.int64)
    col = np.asarray(edge_index[1], np.int64)
    sf = np.asarray(structural_features, np.float32)
    x = np.asarray(x, np.float32)

    rowt = _trow(row).astype(np.int32)
    colt = _trow(col).astype(np.int32)
    bucket = (rowt // CHUNK) * NCHUNKS + (colt // CHUNK)
    border = _bucket_order()
    brank = np.empty(NCHUNKS * NCHUNKS, dtype=np.int64)
    brank[border] = np.arange(NCHUNKS * NCHUNKS)
    nbuck = NCHUNKS * NCHUNKS

    if SORT_MODE == "z":
        rl15 = (rowt % CHUNK).astype(np.int64)
        cl15 = (colt % CHUNK).astype(np.int64)

        def spread(v):
            v = (v | (v << 16)) & 0x0000FFFF0000FFFF
            v = (v | (v << 8)) & 0x00FF00FF00FF00FF
            v = (v | (v << 4)) & 0x0F0F0F0F0F0F0F0F
            v = (v | (v << 2)) & 0x3333333333333333
            v = (v | (v << 1)) & 0x5555555555555555
            return v
        skey = spread(rl15) | (spread(cl15) << 1)
    else:
        skey = rowt
    # global sort by (bucket order, locality key); split each bucket's
    # contiguous run into NCORES near-equal contiguous parts.
    gorder = np.lexsort((skey, brank[bucket]))
    gcnts = np.bincount(bucket, minlength=nbuck)
    goff = np.concatenate([[0], np.cumsum(gcnts[border])])
    cores = []
    cnts = np.zeros((NCORES, nbuck), dtype=np.int64)
    parts = {}
    for i, b in enumerate(border):
        lo, hi = int(goff[i]), int(goff[i + 1])
        cuts = [lo + (hi - lo) * c // NCORES for c in range(NCORES + 1)]
        parts[b] = cuts
        for c in range(NCORES):
            cnts[c, b] = cuts[c + 1] - cuts[c]
    for c in range(NCORES):
        order = np.concatenate(
            [gorder[parts[b][c]:parts[b][c + 1]] for b in border])
        cores.append(order)
    tpb_b = np.maximum(1, -(-cnts.max(axis=0) // GT))           # [nbuck]
    cmax = cnts.max(axis=0)                                     # [nbuck]
    groups = []
    for b in border:
        t = int(tpb_b[b])
        done = 0
        while t > 0:
            gs = min(GSMAX, t)
            nis = []
            for j in range(gs):
                real = min(max(int(cmax[b]) - (done + j) * GT, 0), GT)
                nis.append(max(128, -(-real // 128) * 128))
            groups.append((b, gs, tuple(nis)))
            done += gs
            t -= gs
    ngroups = len(groups)

    in_maps = []
    eid_all = []
    for c in range(NCORES):
        order = cores[c]
        idxr_d = np.zeros((ngroups, 128, GSMAX * IW), dtype=np.int16)
        idxc_d = np.zeros((ngroups, 128, GSMAX * IW), dtype=np.int16)
        sfd_d = np.zeros((ngroups, 128, GSMAX * SL * 4), dtype=np.float32)
        eids_d = np.full((ngroups, GSMAX * GT), -1, dtype=np.int64)
        boff_b = np.concatenate([[0], np.cumsum(cnts[c][border])])
        boff = {b: (boff_b[i], boff_b[i + 1]) for i, b in enumerate(border)}
        tile_done = {b: 0 for b in range(nbuck)}
        for g, (b, gs, _nis) in enumerate(groups):
            t0 = tile_done[b]
            tile_done[b] = t0 + gs
            blo, bhi = boff[b]
            lo = blo + t0 * GT
            hi = min(bhi, lo + gs * GT)
            cnt = max(0, int(hi - lo))
            ids = order[lo:hi]
            npad = gs * GT
            rl = np.zeros(npad, dtype=np.int16)
            cl = np.zeros(npad, dtype=np.int16)
            rl[:cnt] = (rowt[ids] % CHUNK).astype(np.int16)
            cl[:cnt] = (colt[ids] % CHUNK).astype(np.int16)
            eids_d[g, :cnt] = ids
            sfp = np.zeros((npad, 4), dtype=np.float32)
            sfp[:cnt] = sf[ids]

            def wrap(a):
                a = a.reshape(gs, IW, 16).transpose(0, 2, 1)    # [gs, 16, IW]
                a = np.tile(a, (1, 8, 1))                       # [gs, 128, IW]
                return a.transpose(1, 0, 2).reshape(128, gs * IW)
            idxr_d[g, :, :gs * IW] = wrap(rl)
            idxc_d[g, :, :gs * IW] = wrap(cl)
            sfd_d[g, :, :gs * SL * 4] = (
                sfp.reshape(gs, SL, 128, 4).transpose(2, 0, 1, 3)
                .reshape(128, gs * SL * 4))
        in_maps.append({
            "xbf": xbf_host, "w16": w16, "gamma": gamma,
            "idxr": idxr_d, "idxc": idxc_d, "sfd": sfd_d,
        })
        eid_all.append(eids_d)
    return in_maps, eid_all, groups


def unshard(results, eid_all, groups):
    y = np.empty((E, H), dtype=np.float32)
    for c in range(NCORES):
        yd = results[c]["ydev"]          # [ngroups, 128, GSMAX*SL*H]
        eids = eid_all[c]                # [ngroups, GSMAX*GT]
        for g, (b, gs, _nis) in enumerate(groups):
            blk = yd[g, :, :gs * SL * H].reshape(128, gs, SL, H)
            blk = blk.transpose(1, 2, 0, 3).reshape(gs * GT, H)
            ids = eids[g, :gs * GT]
            valid = ids >= 0
            y[ids[valid]] = blk[valid]
    return y


_CACHE = {}


def kernel(**inputs):
    in_maps, eid_all, groups = prep_inputs(**inputs)
    key = tuple(groups)
    if key not in _CACHE:
        _CACHE[key] = build_program(groups)
    nc = _CACHE[key]
    res = run_bass_kernel_spmd(nc, in_maps, core_ids=list(range(NCORES)))
    return unshard(res.results, eid_all, groups)
